# revision 75
# baseline (speedup 1.0000x reference)
"""Trainium2 Bass kernel for ConstraintViolationLoss (GNN message passing).

Two launches on 8 NeuronCores (SPMD), fp16 data streams:

  Launch 1 (softmax expected-value head): logits are laid out class-on-
  partition ([128, ncol] tiles, partition p = 16*g + c holding class c of
  row-group g), ACT computes exp in fp16, and ONE PE matmul against a
  constant [128, 16] weight block produces both softmax sums per row
  (denominator via ones-blocks, numerator via class-value blocks) in PSUM.
  A DRAM bounce regroups the [16, cc] PSUM tile to [128, *] so the DVE
  divide/add runs with all partitions active.

  Host then assembles x (index scatter only), gathers x along the sorted
  edge list, and lays edge (x, feature) pairs out slot-major per
  constraint-degree tier so the per-constraint segment sum becomes a
  binary tree of contiguous fp16 tensor_tensor adds (2x DVE mode).

  Launch 2: per chunk w = xg * ft (fp16, in place), tree-reduce to Ax,
  then one stats pass: viol = relu(Ax - bias), sum / max / count.
"""

import sys

sys.path.insert(0, "/opt/trn_rl_repo")

import numpy as np

import concourse.bass as bass
import concourse.mybir as mybir
from concourse.bass_utils import run_bass_kernel_spmd

P = 128
NCORES = 8
NBINS = P * NCORES
LAMBDA_MEAN, LAMBDA_MAX = 1.0, 0.1
BIAS_COL = 1
LP_SOL_COL = 8
BIG_BIAS = 60000.0          # fp16-safe "never violated" bias for padding segs
CNT_THR = 1e-6
F16 = mybir.dt.float16
F32 = mybir.dt.float32

# phase-1 geometry
P1_CC = 480                 # columns per chunk (rows = 8 per column)
P1_GRP = 3                  # chunks per PSUM group (row offsets 0/32/64)
P1_TW = 120                 # transpose tile width (out partitions)
# phase-2 chunking
CH_TARGET = 3072            # target stream elems / partition / chunk
NBUF2 = 6                   # stream buffers in phase 2
DEBUG_AX = False
P2_NO_POOL = False
MIN_TIER = 6 * NBINS        # merge degree tiers smaller than this

# most recent build params, for the test harness
LAST_ROWS_PP = None
LAST_P2_ARGS = None


# --------------------------------------------------------------------------
# phase 1: expected = (softmax(logits) @ [0..C)) + offsets
# --------------------------------------------------------------------------
def _p1_groups(nch):
    """First group gets the remainder (possibly 1 or 2 chunks) so the
    pipeline starts with a small exp; the rest are full."""
    rem = nch % P1_GRP
    gs = [rem] if rem else [P1_GRP]
    left = nch - gs[0]
    while left > 0:
        gs.append(P1_GRP)
        left -= P1_GRP
    return gs


def _build_phase1(params):
    """Chunk c of group gi writes its [32, cc] matmul output (rows: D_g at
    32t+g, N_g at 32t+8+g) into a shared [128, cc] PSUM tile.  DVE
    evacuates each group to SBUF as fp16, PE transposes 120-column tiles so
    D and N land on the same partition (different free offsets), and DVE
    divides batches of two groups at once."""
    global LAST_ROWS_PP
    LAST_ROWS_PP = params
    nch, cc = params
    gs = _p1_groups(nch)
    ngrp = len(gs)
    cum = np.cumsum(gs).tolist()
    nbatch = _ceil_div(ngrp, 2)
    tw = P1_TW
    ntile = cc // tw            # transpose tiles per group
    dcols = ntile * 2 * 24      # e-columns per group-pair batch
    ncol = nch * cc

    nc = bass.Bass()
    lg = nc.declare_dram_parameter("logits", [P, ncol], F16, isOutput=False)
    wp = nc.declare_dram_parameter("wmat", [P, 32], F16, isOutput=False)
    idp = nc.declare_dram_parameter("ident", [P, 96], F16, isOutput=False)
    op = nc.declare_dram_parameter("offs", [tw, nbatch * dcols], F32, False)
    ex = nc.declare_dram_parameter(
        "expected", [tw, nbatch * dcols], F32, isOutput=True
    )

    nbuf = min(ngrp, 4)
    gdc = ntile * 24            # e-columns per group
    from contextlib import ExitStack

    with ExitStack() as st:
        ec = st.enter_context
        tlg = ec(nc.sbuf_tensor([P, nbuf, P1_GRP * cc], F16))
        te = ec(nc.sbuf_tensor([P, nbuf, P1_GRP * cc], F16))
        tw_ = ec(nc.sbuf_tensor([P, 32], F16))
        tid = ec(nc.sbuf_tensor([P, 96], F16))
        twu = ec(nc.sbuf_tensor([P, 256], F16))        # PE warmup source
        sbc = ec(nc.sbuf_tensor([P, 2, cc], F16))      # psum evacuation
        trec = ec(nc.sbuf_tensor([P, gdc], F32))
        toffs = ec(nc.sbuf_tensor([P, ngrp * gdc], F32))
        ebuf = ec(nc.sbuf_tensor([P, ngrp * gdc], F32))
        ps = [
            ec(nc.psum_tensor(f"ps{i}", [P, cc], F32)) for i in range(2)
        ]
        pt = [
            ec(nc.psum_tensor(f"pt{i}", [P, ntile, 96], F16))
            for i in range(2)
        ]
        pwu = ec(nc.psum_tensor("pwu", [32, 256], F32))
        block = ec(nc.Block())
        wsem = ec(nc.semaphore("wsem"))
        usem = ec(nc.semaphore("usem"))
        lsem = ec(nc.semaphore("lsem"))
        esem = ec(nc.semaphore("esem"))
        msem = ec(nc.semaphore("msem"))
        csem = ec(nc.semaphore("csem"))
        tsem = ec(nc.semaphore("tsem"))
        vsem = ec(nc.semaphore("vsem"))
        osem = ec(nc.semaphore("osem"))

        @block.sync
        def _(sync):
            for gi in range(ngrp):
                if gi >= nbuf:
                    sync.wait_ge(esem, gi - nbuf + 1)    # tlg buffer reuse
                c0 = cum[gi] - gs[gi]
                sync.dma_start(
                    out=tlg[:, gi % nbuf, 0 : gs[gi] * cc],
                    in_=lg[:, c0 * cc : cum[gi] * cc],
                ).then_inc(lsem, 16)
            half_b = nbatch // 2
            sync.wait_ge(vsem, 2 * half_b)
            sync.dma_start(
                out=ex[:, 0 : 2 * half_b * gdc],
                in_=ebuf[0:tw, 0 : 2 * half_b * gdc],
            ).then_inc(osem, 16)
            sync.wait_ge(vsem, ngrp)
            sync.dma_start(
                out=ex[:, 2 * half_b * gdc :],
                in_=ebuf[0:tw, 2 * half_b * gdc :],
            ).then_inc(osem, 16)
            sync.wait_ge(osem, 32)

        @block.scalar
        def _(scalar):
            scalar.dma_start(out=tw_[:], in_=wp[:]).then_inc(wsem, 16)
            scalar.dma_start(out=tid[:], in_=idp[:]).then_inc(wsem, 16)
            scalar.dma_start(out=toffs[0:tw, :], in_=op[:]).then_inc(wsem, 16)
            for gi in range(ngrp):
                scalar.wait_ge(lsem, 16 * (gi + 1))
                if gi >= nbuf:
                    scalar.wait_ge(msem, cum[gi - nbuf])   # te buffer reuse
                nc.scalar.activation(
                    out=te[:, gi % nbuf, 0 : gs[gi] * cc],
                    in_=tlg[:, gi % nbuf, 0 : gs[gi] * cc],
                    func=mybir.ActivationFunctionType.Exp,
                ).then_inc(esem, 1)
        @block.gpsimd
        def _(gpsimd):
            gpsimd.memset(twu[:], 0.0)
            gpsimd.drain().then_inc(usem, 1)

        def _mm_group(tensor, gi):
            tensor.wait_ge(esem, gi + 1)
            if gi >= 2:
                tensor.wait_ge(csem, gi - 1)   # psum tile reuse
            for t in range(gs[gi]):
                nc.tensor.matmul(
                    out=ps[gi % 2][32 * t : 32 * t + 32, :],
                    lhsT=tw_[:],
                    rhs=te[:, gi % nbuf, t * cc : (t + 1) * cc],
                    start=True, stop=True,
                ).then_inc(msem, 1)

        def _tr_group(tensor, gi):
            tensor.wait_ge(csem, gi + 1)
            if gi >= 2:
                tensor.wait_ge(vsem, gi - 1)   # psumT tile reuse
            for k in range(ntile):
                nc.tensor.transpose(
                    out=pt[gi % 2][0:tw, k, :],
                    in_=sbc[0:96, gi % 2, k * tw : (k + 1) * tw],
                    identity=tid[0:96, :],
                ).then_inc(tsem, 1)

        @block.tensor
        def _(tensor):
            # keep the PE busy early so it is at full p-state for real work
            tensor.wait_ge(usem, 1)
            for _ in range(14):
                nc.tensor.matmul(
                    out=pwu[:, :], lhsT=twu[:, 0:32], rhs=twu[:],
                    start=True, stop=True,
                )
            tensor.wait_ge(wsem, 32)
            _mm_group(tensor, 0)
            for gi in range(1, ngrp):
                _mm_group(tensor, gi)
                _tr_group(tensor, gi - 1)
            _tr_group(tensor, ngrp - 1)

        def _div(vector, gi):
            vector.wait_ge(tsem, ntile * (gi + 1))
            pb = pt[gi % 2][0:tw, :, :].rearrange("p a (t m) -> p a t m", m=32)
            sl = slice(gi * gdc, (gi + 1) * gdc)
            nc.vector.reciprocal(out=trec[0:tw, :], in_=pb[:, :, :, 0:8])
            nc.vector.tensor_tensor(
                out=ebuf[0:tw, sl], in0=pb[:, :, :, 8:16],
                in1=trec[0:tw, :], op=mybir.AluOpType.mult,
            )
            nc.vector.tensor_tensor(
                out=ebuf[0:tw, sl], in0=ebuf[0:tw, sl],
                in1=toffs[0:tw, sl], op=mybir.AluOpType.add,
            )
            nc.vector.drain().then_inc(vsem, 1)

        @block.vector
        def _(vector):
            vector.wait_ge(wsem, 48)
            for gi in range(ngrp):
                vector.wait_ge(msem, cum[gi])
                if gi >= 2:
                    vector.wait_ge(tsem, ntile * (gi - 1))   # sbc reuse
                nc.vector.tensor_copy(
                    out=sbc[0:96, gi % 2, :], in_=ps[gi % 2][0:96, :]
                )
                nc.vector.drain().then_inc(csem, 1)
                if gi >= 1:
                    _div(vector, gi - 1)
            _div(vector, ngrp - 1)

    return nc


# --------------------------------------------------------------------------
# phase 2: w = xg*ft, tree segment-sum -> Ax, viol stats
# --------------------------------------------------------------------------
def _build_phase2(tiers):
    """tiers: tuple of (L, Spad, Rc) per degree tier; chunk = Rc ranks.

    DVE runs a skewed pipeline (chunk i's mult/first-halving interleaved
    with chunk i-1's upper stages) so dependent ops are never adjacent;
    drains are inserted automatically only where adjacency remains.
    GPSIMD independently handles a column slice of each big chunk."""
    global LAST_P2_ARGS
    LAST_P2_ARGS = (tiers,)
    nc = bass.Bass()
    ax_tot = sum(s for _, s, _ in tiers)
    chunks = []          # (tier_idx, chunk_idx, axbase)
    axb = 0
    for t, (L, Spad, Rc) in enumerate(tiers):
        for ch in range(Spad // Rc):
            chunks.append((t, ch, axb + ch * Rc))
        axb += Spad
    chmax = max(2 * L * Rc for L, _, Rc in tiers)
    nchunks = len(chunks)

    xs = [
        nc.declare_dram_parameter(f"st{t}", [P, Spad * 2 * L], F16, False)
        for t, (L, Spad, Rc) in enumerate(tiers)
    ]
    bs = nc.declare_dram_parameter("bias", [P, ax_tot], F16, isOutput=False)
    out_p = nc.declare_dram_parameter("partials", [P, 4], F32, isOutput=True)
    axdump = nc.declare_dram_parameter("axdump", [P, ax_tot], F16, isOutput=True) if DEBUG_AX else None

    with (
        nc.sbuf_tensor([P, NBUF2, chmax], F16) as tst,
        nc.sbuf_tensor([P, ax_tot], F16) as tax,
        nc.sbuf_tensor([P, ax_tot], F16) as tb,
        nc.sbuf_tensor([P, ax_tot], F16) as tv,
        nc.sbuf_tensor([P, 12], F16) as tm16,
        nc.sbuf_tensor([P, 12], F32) as tsum,
        nc.sbuf_tensor([P, 12], F32) as tcnt,
        nc.sbuf_tensor([P, 4], F32) as tout,
        nc.Block() as block,
        nc.semaphore("bsem") as bsem,
        nc.semaphore("pa") as pa,
        nc.semaphore("g2") as g2,
        nc.semaphore("vs") as vs,
        nc.semaphore("fsem") as fsem,
        nc.semaphore("osem") as osem,
    ):

        def _l1_split(i):
            """Pool owns the last z columns of the first halving: it
            computes both products it needs and the halving add itself,
            gated only on the chunk DMA.  Returns (m, hf, z)."""
            t, ch, axb_c = chunks[i]
            L, Spad, Rc = tiers[t]
            assert L % 2 == 0
            m = L * Rc
            hf = (L // 2) * Rc
            if P2_NO_POOL or 2 * m < 2000:
                return m, hf, 0
            z = min(240, hf // 3 // 16 * 16)
            return m, hf, z

        @block.sync
        def _(sync):
            for i, (t, ch, _axb) in enumerate(chunks):
                L, Spad, Rc = tiers[t]
                sz = 2 * L * Rc
                if i >= NBUF2:
                    sync.wait_ge(vs, i - NBUF2 + 1)
                sync.dma_start(
                    out=tst[:, i % NBUF2, 0:sz],
                    in_=xs[t][:, ch * sz : (ch + 1) * sz],
                ).then_inc(pa, 16)
            sync.wait_ge(fsem, 1)
            sync.dma_start(out=out_p[:], in_=tout[:]).then_inc(osem, 16)
            if DEBUG_AX:
                sync.dma_start(out=axdump[:], in_=tax[:]).then_inc(osem, 16)
                sync.wait_ge(osem, 32)
            else:
                sync.wait_ge(osem, 16)

        @block.scalar
        def _(scalar):
            scalar.dma_start(out=tb[:], in_=bs[:]).then_inc(bsem, 16)

        @block.gpsimd
        def _(gpsimd):
            for i, (t, ch, axb_c) in enumerate(chunks):
                m, hf, z = _l1_split(i)
                w = tst[:, i % NBUF2, :]
                if z == 0:
                    gpsimd.sem_inc(g2, 1)
                    continue
                gpsimd.wait_ge(pa, 16 * (i + 1))
                nc.gpsimd.tensor_tensor(
                    out=w[0:P, hf - z : hf], in0=w[0:P, hf - z : hf],
                    in1=w[0:P, m + hf - z : m + hf],
                    op=mybir.AluOpType.mult,
                )
                nc.gpsimd.tensor_tensor(
                    out=w[0:P, m - z : m], in0=w[0:P, m - z : m],
                    in1=w[0:P, 2 * m - z : 2 * m],
                    op=mybir.AluOpType.mult,
                )
                gpsimd.drain()
                nc.gpsimd.tensor_tensor(
                    out=w[0:P, hf - z : hf], in0=w[0:P, hf - z : hf],
                    in1=w[0:P, m - z : m], op=mybir.AluOpType.add,
                )
                gpsimd.drain().then_inc(g2, 1)

        # ---- DVE skewed pipeline ----
        last_key = [None]

        def emit(fn, dep, key):
            if dep is not None and last_key[0] == dep:
                nc.vector.drain()
            fn()
            last_key[0] = key

        def mk_mults(i):
            m, hf, z = _l1_split(i)
            w = tst[:, i % NBUF2, :]
            ops = []
            if z == 0:
                def f0(w=w, m=m):
                    nc.vector.tensor_tensor(
                        out=w[0:P, 0:m], in0=w[0:P, 0:m],
                        in1=w[0:P, m : 2 * m], op=mybir.AluOpType.mult,
                    )
                ops.append((f0, None, (i, "mult")))
            else:
                def fa(w=w, m=m, hf=hf, z=z):
                    nc.vector.tensor_tensor(
                        out=w[0:P, 0 : hf - z], in0=w[0:P, 0 : hf - z],
                        in1=w[0:P, m : m + hf - z], op=mybir.AluOpType.mult,
                    )
                def fb(w=w, m=m, hf=hf, z=z):
                    nc.vector.tensor_tensor(
                        out=w[0:P, hf : m - z], in0=w[0:P, hf : m - z],
                        in1=w[0:P, m + hf : 2 * m - z],
                        op=mybir.AluOpType.mult,
                    )
                ops.append((fa, None, (i, "mult")))
                ops.append((fb, None, (i, "mult")))
            return ops

        def mk_l1(i):
            m, hf, z = _l1_split(i)
            w = tst[:, i % NBUF2, :]
            def f(w=w, m=m, hf=hf, z=z):
                nc.vector.tensor_tensor(
                    out=w[0:P, 0 : hf - z], in0=w[0:P, 0 : hf - z],
                    in1=w[0:P, hf : m - z], op=mybir.AluOpType.add,
                )
            return (f, (i, "mult"), (i, "l1"))

        def mk_upper(i):
            t, ch, axb_c = chunks[i]
            L, Spad, Rc = tiers[t]
            w = tst[:, i % NBUF2, :]
            stages = []
            h = L // 2
            prev = (i, "l1")
            k = 0
            while h % 2 == 0 and h > 2 and h * Rc > 460:
                hf2 = (h // 2) * Rc
                def fh(w=w, hf2=hf2):
                    nc.vector.tensor_tensor(
                        out=w[0:P, 0:hf2], in0=w[0:P, 0:hf2],
                        in1=w[0:P, hf2 : 2 * hf2], op=mybir.AluOpType.add,
                    )
                stages.append((fh, prev, (i, f"h{k}")))
                prev = (i, f"h{k}")
                k += 1
                h //= 2
            def fr(w=w, h=h, Rc=Rc, axb_c=axb_c):
                src = w[0:P, 0 : h * Rc].rearrange("p (s r) -> p r s", r=Rc)
                with nc.allow_low_precision(reason="fp16 segment sums"):
                    nc.vector.tensor_reduce(
                        out=tax[:, axb_c : axb_c + Rc], in_=src,
                        axis=mybir.AxisListType.X, op=mybir.AluOpType.add,
                    ).then_inc(vs, 1)
            stages.append((fr, prev, (i, "red")))
            return stages

        def _piece(pi, lo, hi):
            nc.vector.drain()
            nc.vector.tensor_tensor(
                out=tv[:, lo:hi], in0=tax[:, lo:hi], in1=tb[:, lo:hi],
                op=mybir.AluOpType.subtract,
            )
            nc.vector.drain()
            nc.vector.tensor_scalar_max(
                out=tv[:, lo:hi], in0=tv[:, lo:hi], scalar1=0.0
            )
            nc.vector.drain()
            nc.vector.tensor_reduce(
                out=tsum[:, pi : pi + 1], in_=tv[:, lo:hi],
                axis=mybir.AxisListType.X, op=mybir.AluOpType.add,
            )
            nc.vector.tensor_reduce(
                out=tm16[:, pi : pi + 1], in_=tv[:, lo:hi],
                axis=mybir.AxisListType.X, op=mybir.AluOpType.max,
            )
            nc.vector.tensor_scalar(
                out=tv[:, lo:hi], in0=tv[:, lo:hi], scalar1=CNT_THR,
                scalar2=None, op0=mybir.AluOpType.is_gt,
            )
            nc.vector.drain()
            nc.vector.tensor_reduce(
                out=tcnt[:, pi : pi + 1], in_=tv[:, lo:hi],
                axis=mybir.AxisListType.X, op=mybir.AluOpType.add,
            )
            last_key[0] = None

        @block.vector
        def _(vector):
            pend = []
            piece_lo = [0]
            npieces = [0]
            bwait = [False]

            def maybe_piece(prefix, force=False):
                if prefix - piece_lo[0] <= 0:
                    return
                if not force and prefix - piece_lo[0] < ax_tot // 6:
                    return
                if not bwait[0]:
                    vector.wait_ge(bsem, 16)
                    bwait[0] = True
                _piece(npieces[0], piece_lo[0], prefix)
                piece_lo[0] = prefix
                npieces[0] += 1

            for i in range(nchunks):
                if i >= 4:
                    maybe_piece(chunks[i - 1][2])
                vector.wait_ge(pa, 16 * (i + 1))
                for op in mk_mults(i):
                    emit(*op)
                if pend:
                    vector.wait_ge(g2, i)
                    emit(*pend[0])
                emit(*mk_l1(i))
                for op in pend[1:]:
                    emit(*op)
                pend = mk_upper(i)
            vector.wait_ge(g2, nchunks)
            for op in pend:
                emit(*op)
            maybe_piece(ax_tot, force=True)
            # combine the pieces
            np_ = npieces[0]
            nc.vector.drain()
            nc.vector.tensor_reduce(
                out=tout[:, 0:1], in_=tsum[:, 0:np_],
                axis=mybir.AxisListType.X, op=mybir.AluOpType.add,
            )
            nc.vector.tensor_reduce(
                out=tout[:, 1:2], in_=tm16[:, 0:np_],
                axis=mybir.AxisListType.X, op=mybir.AluOpType.max,
            )
            nc.vector.tensor_reduce(
                out=tout[:, 2:3], in_=tcnt[:, 0:np_],
                axis=mybir.AxisListType.X, op=mybir.AluOpType.add,
            )
            nc.vector.drain()
            nc.vector.tensor_copy(out=tout[:, 3:4], in_=tout[:, 2:3])
            nc.vector.drain().then_inc(fsem, 1)

    return nc


def _ceil_div(a, b):
    return -(-a // b)


# --------------------------------------------------------------------------
# host-side layout prep (index shuffling only)
# --------------------------------------------------------------------------
def _p1_rowmap(nch, cc):
    """row_of[j, bi, ti, t, g] -> packed row index (or -1 if the slot is
    junk), matching the device's e/offs column order col = bi*dcols +
    ti*24 + t*8 + g with partition j."""
    gs = _p1_groups(nch)
    ngrp = len(gs)
    nbatch = _ceil_div(ngrp, 2)
    tw = P1_TW
    ntile = cc // tw
    ncol = nch * cc
    j, bi, ti, t, g = np.meshgrid(
        np.arange(tw), np.arange(nbatch), np.arange(2 * ntile),
        np.arange(P1_GRP), np.arange(8), indexing="ij",
    )
    gi = bi * 2 + ti // ntile
    k4 = ti % ntile
    valid = (gi < ngrp) & (t < np.asarray(gs + [0])[np.minimum(gi, ngrp)])
    chunk = np.cumsum([0] + gs)[np.minimum(gi, ngrp - 1)] + t
    row = g * ncol + chunk * cc + k4 * tw + j
    row = np.where(valid, row, -1)
    return row, ncol


def _prep_phase1(logits, offsets):
    """Pack per-core class-major logits grids + offsets; return arrays."""
    ns, ccls = logits.shape
    assert ccls == 16
    rows_core = _ceil_div(ns, NCORES)
    ncol_need = _ceil_div(rows_core, 8)
    cc = P1_CC
    nch = _ceil_div(ncol_need, cc)
    ncol = nch * cc
    rows_cap = 8 * ncol
    tw = P1_TW

    # weight block: cols 0..7 ones per group, 8..15 class values, 16..31 zero
    W = np.zeros((P, 32), dtype=np.float16)
    pidx = np.arange(P)
    g = pidx // 16
    c = pidx % 16
    W[pidx, g] = 1.0
    W[pidx, 8 + g] = c.astype(np.float16)
    ident = np.zeros((P, 96), dtype=np.float16)
    ident[np.arange(96), np.arange(96)] = 1.0

    row_of, _ = _p1_rowmap(nch, cc)
    flat = row_of.reshape(tw, -1)

    lgs, offs_packed = [], []
    for core in range(NCORES):
        lo, hi = core * rows_core, min((core + 1) * rows_core, ns)
        lgp = np.zeros((rows_cap, ccls), dtype=np.float16)
        lgp[: hi - lo] = logits[lo:hi].astype(np.float16)
        # partition p = g*16 + cls, column j; row r = g*ncol + j
        lgs.append(
            np.ascontiguousarray(
                lgp.reshape(8, ncol, ccls).transpose(0, 2, 1).reshape(P, ncol)
            )
        )
        ofp = np.zeros(rows_cap + 1, dtype=np.float32)
        ofp[: hi - lo] = offsets[lo:hi]
        offs_packed.append(np.ascontiguousarray(ofp[flat]))
    return (nch, cc), W, ident, lgs, offs_packed, rows_core, rows_cap


def _unpack_expected(e_packed, nch, cc, rows_cap, nrows):
    row_of, _ = _p1_rowmap(nch, cc)
    flat = row_of.reshape(-1)
    ok = flat >= 0
    out = np.zeros(rows_cap, dtype=np.float32)
    out[flat[ok]] = e_packed.reshape(-1)[ok]
    return out[:nrows]


def _prep_phase2(con, var, feat, bias, n_con):
    """Sort edges, tier by degree, slot-major layout. Returns metadata +
    per-tier (core-major) index/feature arrays; xg filled later."""
    ne = con.shape[0]
    deg = np.bincount(con, minlength=n_con)
    order = np.argsort(con, kind="stable")
    run_start = np.zeros(n_con + 1, dtype=np.int64)
    np.cumsum(deg, out=run_start[1:])
    con_sorted = con[order]
    off_in_run = np.arange(ne, dtype=np.int64) - run_start[con_sorted]
    var_sorted = var[order]
    feat_sorted = feat[order]

    maxdeg = int(deg.max()) if ne else 1
    cand = list(range(16, 68, 4))
    if maxdeg > cand[-1]:
        cand.append(_ceil_div(maxdeg, 4) * 4)
    cand = np.asarray(cand, dtype=np.int64)
    t_cand = np.searchsorted(cand, deg, side="left")
    cnt = np.bincount(t_cand, minlength=len(cand))
    # merge small tiers upward into the next stride
    keep = []
    acc = 0
    remap = np.zeros(len(cand), dtype=np.int64)
    for si in range(len(cand)):
        acc += cnt[si]
        remap[si] = len(keep)
        if (acc >= MIN_TIER) or (si == len(cand) - 1 and acc > 0):
            keep.append(int(cand[si]))
            acc = 0
    t_of_seg = remap[t_cand]

    raw = []
    for t, L in enumerate(keep):
        segs = np.nonzero(t_of_seg == t)[0]
        n_t = segs.shape[0]
        if n_t == 0:
            continue
        S_t = _ceil_div(n_t, NBINS)
        nch_t = max(1, int(round(S_t * 2 * L / CH_TARGET)))
        nch_t = min(nch_t, S_t)
        Rc = _ceil_div(S_t, nch_t)
        Spad = nch_t * Rc
        raw.append((t, L, Spad, Rc, segs))
    # processing order = tier order: small tier first and last (short
    # pipeline fill/drain), big tiers in the middle
    order = sorted(range(len(raw)), key=lambda i: -raw[i][2] * raw[i][1])
    if len(order) >= 3:
        order = [order[-2]] + order[:-2] + [order[-1]]

    tiers = []
    tier_data = []
    axb = 0
    dense = np.full(len(keep), -1, dtype=np.int64)
    for i in order:
        t, L, Spad, Rc, segs = raw[i]
        dense[t] = len(tiers)
        k_of_con = np.full(n_con, -1, dtype=np.int64)
        k_of_con[segs] = np.arange(segs.shape[0])
        tiers.append((L, Spad, Rc))
        tier_data.append((segs, k_of_con, axb))
        axb += Spad
    t_of_seg = dense[t_of_seg]

    ax_tot = axb
    bias_arr = np.full((NCORES, P, ax_tot), BIG_BIAS, dtype=np.float16)
    for (L, Spad, Rc), (segs, k_of_con, axb) in zip(tiers, tier_data):
        k = k_of_con[segs]
        bb = k % NBINS
        r = k // NBINS
        bias_arr[bb // P, bb % P, axb + r] = bias[segs].astype(np.float16)

    return (
        tiers,
        tier_data,
        ax_tot,
        bias_arr,
        con_sorted,
        off_in_run,
        var_sorted,
        feat_sorted,
        t_of_seg,
    )


def _fill_streams(tiers, tier_data, t_of_seg, con_sorted, off_in_run,
                  var_sorted, feat_sorted, x16):
    """Build per-tier interleaved (xg, ft) fp16 streams, slot-major."""
    e_tier = t_of_seg[con_sorted]
    streams = []
    for t, ((L, Spad, Rc), (segs, k_of_con, axb)) in enumerate(
        zip(tiers, tier_data)
    ):
        sel = np.nonzero(e_tier == t)[0]
        cs = con_sorted[sel]
        slot = off_in_run[sel]
        k = k_of_con[cs]
        b = k % NBINS
        r = k // NBINS
        core = b // P
        part = b % P
        ch = r // Rc
        rin = r % Rc
        base = ch * (2 * L * Rc)
        col_x = base + slot * Rc + rin
        col_f = base + (L + slot) * Rc + rin
        width = Spad * 2 * L
        arr = np.zeros(NCORES * P * width, dtype=np.float16)
        flat_base = (core * P + part) * width
        arr[flat_base + col_x] = x16[var_sorted[sel]]
        arr[flat_base + col_f] = feat_sorted[sel].astype(np.float16)
        streams.append(arr.reshape(NCORES, P, width))
    return streams


# --------------------------------------------------------------------------
def kernel(**inputs) -> tuple:
    prob_bin = np.asarray(inputs["prob_bin"], dtype=np.float32)
    logits = np.asarray(inputs["logits_int_small"], dtype=np.float32)
    offsets = np.asarray(inputs["int_small_offsets"], dtype=np.float32)
    pred_l = np.asarray(inputs["pred_int_large"], dtype=np.float32)
    feat = np.asarray(inputs["edge_features"], dtype=np.float32).reshape(-1)
    cfeat = np.asarray(inputs["constraint_features"], dtype=np.float32)
    vfeat = np.asarray(inputs["variable_features"], dtype=np.float32)
    idx_bin = np.asarray(inputs["idx_bin"], dtype=np.int64)
    idx_s = np.asarray(inputs["idx_int_small"], dtype=np.int64)
    idx_l = np.asarray(inputs["idx_int_large"], dtype=np.int64)
    var_types = np.asarray(inputs["var_types"], dtype=np.int64)
    ei = np.asarray(inputs["edge_indices"], dtype=np.int64)
    n_vars = int(inputs["n_vars"])

    n_con = cfeat.shape[0]
    ns = logits.shape[0]
    bias = np.ascontiguousarray(cfeat[:, BIAS_COL])
    lp_vals = np.ascontiguousarray(vfeat[:, LP_SOL_COL])

    # ---------------- launch 1 ----------------
    (nch, cc), W, ident, lgs, offs_packed, rows_core, rows_cap = _prep_phase1(
        logits, offsets
    )
    nc1 = _build_phase1((nch, cc))
    in1 = [
        {"logits": lgs[c], "wmat": W, "ident": ident, "offs": offs_packed[c]}
        for c in range(NCORES)
    ]
    res1 = run_bass_kernel_spmd(nc1, in1, list(range(NCORES)))
    expected = np.concatenate(
        [
            _unpack_expected(
                res1.results[c]["expected"], nch, cc, rows_cap, rows_core
            )
            for c in range(NCORES)
        ]
    )[:ns]

    # ---------------- host: assemble x ----------------
    xfull = np.zeros(n_vars, dtype=np.float32)
    xfull[idx_bin] = prob_bin[:, 0]
    xfull[idx_s] = expected
    xfull[idx_l] = pred_l[:, 0]
    xfull = np.where(var_types == 0, lp_vals, xfull)
    x16 = xfull.astype(np.float16)

    # ---------------- launch 2 ----------------
    (
        tiers, tier_data, ax_tot, bias_arr, con_sorted, off_in_run,
        var_sorted, feat_sorted, t_of_seg,
    ) = _prep_phase2(ei[0], ei[1], feat, bias, n_con)
    streams = _fill_streams(
        tiers, tier_data, t_of_seg, con_sorted, off_in_run, var_sorted,
        feat_sorted, x16,
    )
    nc2 = _build_phase2(tuple(tiers))
    in2 = []
    for c in range(NCORES):
        m = {f"st{t}": streams[t][c] for t in range(len(tiers))}
        m["bias"] = bias_arr[c]
        in2.append(m)
    res2 = run_bass_kernel_spmd(nc2, in2, list(range(NCORES)))

    parts = np.stack([res2.results[c]["partials"] for c in range(NCORES)])
    vsum = parts[:, :, 0].astype(np.float64).sum()
    vmax = np.float32(parts[:, :, 1].max())
    vcnt = np.int64(round(float(parts[:, :, 2].astype(np.float64).sum())))
    mean_viol = np.float32(vsum / np.float64(n_con))
    penalty = np.float32(
        np.float32(LAMBDA_MEAN) * mean_viol + np.float32(LAMBDA_MAX) * vmax
    )
    return penalty, mean_viol, vmax, vcnt
